# revision 4
# baseline (speedup 1.0000x reference)
"""Trainium2 Bass kernel for nn_EvolvingSystem (moe_routing).

Computes, for B=2048, K=64 clusters, D=128, R1=1024, OUT=IN=512:
    psi   = softmax(-clamp(d^T S d, 0), axis=K)   d = mu_k - z_b, S = sig sig^T
    y_con   = sum_k psi[b,k]    * |u_b @ W_con[k].T|
    x_recon = sum_k member[b,k] * |u_b @ W_recon[k].T|

Strategy: expert-parallel over 8 NeuronCores (8 clusters per core), partial
outputs summed on host. Per core:
  - u^T resident in SBUF (rounded to float32r), weights streamed per-cluster
    and rounded to float32r; the grouped GEMMs run on the PE at 1 cyc/row.
  - psi computed on-device in plain fp32 via the diagonal-Mahalanobis
    expansion  d2[b,k] = sum_d s[k,d] * (z[b,d] - mu[k,d])^2
             = z2.s_k - 2 z.(s_k*mu_k) + sum_d s_k mu_k^2
    (one (B x 2D+1) @ (2D+1 x K) GEMM + softmax).  Works because S is
    diagonal at runtime; a host fallback encodes a host-computed d2 into the
    same GEMM when S is not diagonal.
  - epilogue per (cluster, output, batch-tile): ACT computes |psum * w| with a
    per-partition scale (w = psi or member column), DVE accumulates.
  - batch processed in 2 half-blocks so accumulators + weight buffers fit SBUF.

The module is self-contained: hardcoded shapes, no file reads.
"""

import os
import sys

import numpy as np

for _p in ("/opt/trn_rl_repo", "/root/.axon_site/_ro/trn_rl_repo"):
    if os.path.isdir(_p) and _p not in sys.path:
        sys.path.insert(0, _p)

import concourse.bass as bass  # noqa: E402
import concourse.tile as tile  # noqa: E402
from concourse import bacc, mybir  # noqa: E402
from concourse.bass_utils import run_bass_kernel_spmd  # noqa: E402

f32 = mybir.dt.float32
f32r = mybir.dt.float32r

B = 2048
K = 64
D = 128
R1 = 1024
OUT = 512
NCORES = 8
KL = K // NCORES      # local clusters per core
BT = B // 128         # batch tiles
RT = R1 // 128        # contraction tiles
NB = 2                # batch half-blocks
BTH = BT // NB        # batch tiles per half-block

_prog_cache = {}
last_exec_time_ns = None


def _build_program(mm_mode: str):
    """One SPMD program, identical on all 8 cores."""
    nc = bacc.Bacc()
    ut_d = nc.declare_dram_parameter("ut", [R1, B], f32, isOutput=False)
    w_d = nc.declare_dram_parameter("wstk", [2 * KL, R1, OUT], f32, isOutput=False)
    zsq_d = nc.declare_dram_parameter("zsq", [D, B], f32, isOutput=False)
    zlin_d = nc.declare_dram_parameter("zlin", [D, B], f32, isOutput=False)
    qs_d = nc.declare_dram_parameter("qs", [D, K], f32, isOutput=False)
    qb_d = nc.declare_dram_parameter("qb", [D, K], f32, isOutput=False)
    qc_d = nc.declare_dram_parameter("qc", [1, K], f32, isOutput=False)
    mem_d = nc.declare_dram_parameter("meml", [B, KL], f32, isOutput=False)
    y_d = nc.declare_dram_parameter("ypart", [B, OUT], f32, isOutput=True)
    x_d = nc.declare_dram_parameter("xpart", [B, OUT], f32, isOutput=True)

    mmdt = f32r if mm_mode == "fp32r" else f32
    Abs = mybir.ActivationFunctionType.Abs
    Exp = mybir.ActivationFunctionType.Exp
    X = mybir.AxisListType.X
    AO = mybir.AluOpType

    with tile.TileContext(nc) as tc:
        with (
            tc.tile_pool(name="consts", bufs=1) as consts,
            tc.tile_pool(name="zpool", bufs=1) as zpool,
            tc.tile_pool(name="wstage", bufs=2) as wstage,
            tc.tile_pool(name="wchunk", bufs=2) as wchunk,
            tc.tile_pool(name="accp", bufs=1) as accp,
            tc.tile_pool(name="tpool", bufs=4) as tpool,
            tc.tile_pool(name="smp", bufs=2) as smp,
            tc.tile_pool(name="psum", bufs=6, space="PSUM") as psum,
            tc.tile_pool(name="psum2", bufs=2, space="PSUM") as psum2,
        ):
            # ---- u^T: stage, round to mm dtype, keep resident ----
            ut_sb = []
            for r in range(RT):
                stg = wstage.tile([128, RT * OUT], f32, tag="wstg")
                nc.sync.dma_start(out=stg[:, 0:B],
                                  in_=ut_d[r * 128:(r + 1) * 128, :])
                t = consts.tile([128, B], mmdt, tag=f"ut{r}")
                nc.vector.tensor_copy(t, stg[:, 0:B])
                ut_sb.append(t)

            # ---- psi inputs ----
            zsq_sb = zpool.tile([128, B], f32, tag="zsq")
            nc.sync.dma_start(out=zsq_sb, in_=zsq_d[:, :])
            zlin_sb = zpool.tile([128, B], f32, tag="zlin")
            nc.sync.dma_start(out=zlin_sb, in_=zlin_d[:, :])
            qs_sb = consts.tile([128, K], f32, tag="qs")
            nc.sync.dma_start(out=qs_sb, in_=qs_d[:, :])
            qb_sb = consts.tile([128, K], f32, tag="qb")
            nc.sync.dma_start(out=qb_sb, in_=qb_d[:, :])
            qc_sb = consts.tile([1, K], f32, tag="qc")
            nc.sync.dma_start(out=qc_sb, in_=qc_d[:, :])
            ones_sb = consts.tile([1, 128], f32, tag="ones")
            nc.vector.memset(ones_sb, 1.0)

            mem_sb = []
            for bt in range(BT):
                m = consts.tile([128, KL], f32, tag=f"mem{bt}")
                nc.sync.dma_start(out=m, in_=mem_d[bt * 128:(bt + 1) * 128, :])
                mem_sb.append(m)

            # ---- psi: d2 GEMM + clamp + softmax over K (plain fp32) ----
            psi_sb = []
            for bt in range(BT):
                bs = slice(bt * 128, (bt + 1) * 128)
                pd = psum2.tile([128, K], f32, tag="pd")
                nc.tensor.matmul(pd, lhsT=zsq_sb[:, bs], rhs=qs_sb[:],
                                 start=True, stop=False)
                nc.tensor.matmul(pd, lhsT=zlin_sb[:, bs], rhs=qb_sb[:],
                                 start=False, stop=False)
                nc.tensor.matmul(pd, lhsT=ones_sb[:], rhs=qc_sb[:],
                                 start=False, stop=True)
                d2 = smp.tile([128, K], f32, tag="d2")
                nc.vector.tensor_scalar_max(d2, pd, 0.0)
                mn = smp.tile([128, 1], f32, tag="mn")
                nc.vector.tensor_reduce(mn, d2, axis=X, op=AO.min)
                e = smp.tile([128, K], f32, tag="e")
                nc.scalar.activation(e, d2, Exp, bias=mn, scale=-1.0)
                sm = smp.tile([128, 1], f32, tag="sm")
                nc.vector.tensor_reduce(sm, e, axis=X, op=AO.add)
                rc = smp.tile([128, 1], f32, tag="rc")
                nc.vector.reciprocal(rc, sm)
                p = consts.tile([128, KL], f32, tag=f"psi{bt}")
                nc.vector.tensor_scalar_mul(p, e[:, 0:KL], rc)
                psi_sb.append(p)

            # ---- main grouped GEMM ----
            for half in range(NB):
                bts = list(range(half * BTH, (half + 1) * BTH))
                accs = {}
                for j in range(2):
                    for bt in bts:
                        accs[(j, bt)] = accp.tile(
                            [128, OUT], f32, name=f"acc{j}_{bt % BTH}",
                            tag=f"acc{j}_{bt % BTH}")
                for i in range(2 * KL):
                    kl, j = divmod(i, 2)
                    stg = wstage.tile([128, RT * OUT], f32, tag="wstg")
                    for r in range(RT):
                        nc.sync.dma_start(
                            out=stg[:, r * OUT:(r + 1) * OUT],
                            in_=w_d[i, r * 128:(r + 1) * 128, :])
                    if mm_mode == "fp32r":
                        wt = wchunk.tile([128, RT * OUT], f32r, tag="wch")
                        nc.vector.tensor_copy(wt, stg[:])
                    else:
                        wt = stg
                    for bt in bts:
                        bs = slice(bt * 128, (bt + 1) * 128)
                        ps = psum.tile([128, OUT], f32, tag="ps")
                        for r in range(RT):
                            nc.tensor.matmul(ps, lhsT=ut_sb[r][:, bs],
                                             rhs=wt[:, r * OUT:(r + 1) * OUT],
                                             start=(r == 0), stop=(r == RT - 1))
                        wv = (psi_sb[bt] if j == 0 else mem_sb[bt])[:, kl:kl + 1]
                        a = accs[(j, bt)]
                        if kl == 0:
                            nc.scalar.activation(a, ps, Abs, scale=wv)
                        else:
                            t = tpool.tile([128, OUT], f32, tag="t")
                            nc.scalar.activation(t, ps, Abs, scale=wv)
                            nc.vector.tensor_add(a, a, t)
                        if kl == KL - 1:
                            od = y_d if j == 0 else x_d
                            nc.sync.dma_start(out=od[bs, :], in_=a)
    nc.finalize()
    return nc


def _get_program(mm_mode: str):
    if mm_mode not in _prog_cache:
        _prog_cache[mm_mode] = _build_program(mm_mode)
    return _prog_cache[mm_mode]


def kernel(z, u, member, mu, sigma_inv, W_con, W_recon):
    global last_exec_time_ns
    z = np.asarray(z, dtype=np.float32)
    u = np.asarray(u, dtype=np.float32)
    member = np.asarray(member, dtype=np.float32)
    mu = np.asarray(mu, dtype=np.float32)
    sigma_inv = np.asarray(sigma_inv, dtype=np.float32)
    W_con = np.asarray(W_con, dtype=np.float32)
    W_recon = np.asarray(W_recon, dtype=np.float32)

    z2 = z[:, 0, :]          # (B, D)
    u2 = u[:, 0, :]          # (B, R1)
    mem2 = member[:, 0, :]   # (B, K)

    # ---- host prep: psi GEMM encoding ----
    S = np.einsum("kde,kfe->kdf", sigma_inv.astype(np.float64),
                  sigma_inv.astype(np.float64))  # (K, D, D)
    s_diag = np.einsum("kdd->kd", S)             # (K, D)
    off = S - s_diag[:, None, :] * np.eye(D, dtype=np.float64)[None]
    diag_ok = np.abs(off).max() <= 1e-5 * (np.abs(s_diag).max() + 1e-30)

    zsq = np.ascontiguousarray((z2 ** 2).T)       # (D, B)
    zlin = np.ascontiguousarray(z2.T)             # (D, B)
    ut = np.ascontiguousarray(u2.T)               # (R1, B)

    if diag_ok:
        qs_full = s_diag.T.astype(np.float32)                      # (D, K)
        qb_full = (-2.0 * s_diag * mu.astype(np.float64)).T.astype(np.float32)
        qc_full = np.einsum("kd,kd->k", s_diag,
                            (mu.astype(np.float64) ** 2)).astype(np.float32)
    else:
        # general fallback: compute d2 on host (fp32, mirroring the
        # reference), then encode d2 into the same device GEMM via
        # zlin^T @ qb with qb = [I_K; 0].
        d = mu[None, :, :] - z2[:, None, :]                        # (B, K, D)
        w = np.einsum("bkd,kde->bke", d, sigma_inv)
        d2_host = np.einsum("bke,bke->bk", w, w).astype(np.float32)
        zsq = np.zeros_like(zsq)
        zlin = np.zeros((D, B), dtype=np.float32)
        qs_full = np.zeros((D, K), dtype=np.float32)
        qb_full = np.zeros((D, K), dtype=np.float32)
        qc_full = np.zeros((K,), dtype=np.float32)

    mm_mode = os.environ.get("BASSK_MM", "fp32r")
    trace = os.environ.get("BASSK_TRACE", "0") == "1"
    nc = _get_program(mm_mode)

    in_maps = []
    for c in range(NCORES):
        local = np.arange(c * KL, (c + 1) * KL)
        others = np.array([k for k in range(K) if k not in set(local)])
        perm = np.concatenate([local, others])
        if diag_ok:
            qs = np.ascontiguousarray(qs_full[:, perm])
            qb = np.ascontiguousarray(qb_full[:, perm])
            qc = np.ascontiguousarray(qc_full[perm][None, :])
            zl = zlin
        else:
            qb = np.zeros((D, K), dtype=np.float32)
            qb[:K, :] = np.eye(K, dtype=np.float32)
            qs = qs_full
            qc = qc_full[None, :].copy()
            zl = np.zeros((D, B), dtype=np.float32)
            zl[:K, :] = d2_host[:, perm].T
        wstk = np.empty((2 * KL, R1, OUT), dtype=np.float32)
        for kli, kg in enumerate(local):
            wstk[2 * kli + 0] = W_con[kg].T
            wstk[2 * kli + 1] = W_recon[kg].T
        in_maps.append({
            "ut": ut,
            "wstk": wstk,
            "zsq": zsq,
            "zlin": np.ascontiguousarray(zl),
            "qs": qs,
            "qb": qb,
            "qc": qc,
            "meml": np.ascontiguousarray(mem2[:, local]),
        })

    br = run_bass_kernel_spmd(nc, in_maps, list(range(NCORES)), trace=trace)
    last_exec_time_ns = br.exec_time_ns

    y = np.zeros((B, OUT), dtype=np.float64)
    x = np.zeros((B, OUT), dtype=np.float64)
    for c in range(NCORES):
        y += br.results[c]["ypart"].astype(np.float64)
        x += br.results[c]["xpart"].astype(np.float64)
    y_con = y.astype(np.float32)[:, None, :]
    x_recon = x.astype(np.float32)[:, None, :]
    return (y_con, x_recon)
